# revision 50
# baseline (speedup 1.0000x reference)
"""Trainium2 Bass kernel for channel (cross-covariance) self-attention.

Shapes (hardcoded): x (8, 4096, 512) f32, wqkv_w (1536, 512), wqkv_b (1536,),
wp_w (512, 512), wp_b (512,). NUM_HEADS=8, head_dim=64.

Sharding: data-parallel over batch b across the 8 NeuronCores (one batch
element per core). Weights replicated (pre-transposed + bf16-cast on host).

Per-core algorithm (bf16 data path, fp32 PSUM accumulation):
  - x is host-cast to bf16; per head block, xT[c, t] tiles are produced by
    dma_start_transpose (HBM -> SBUF via the xbar transpose engine), so the
    PE does no transposes and ACT does no xT evacuations.
  - qk[t, f] (f in 0:1024) via stationary xT tiles against moving bf16
    weights, evacuated in [128,1024] PSUM pairs with fused bias add + bf16
    cast on DVE.
  - scores_h[d,e] = sum_{i,s} q_s^T k_s in bf16 (single-pass matmuls).
  - vT[vc, t] via stationary bf16 weight slices, ACT evac (bias, bf16 cast).
  - y via block-diag(W_h^T) bf16 stationary against vT tiles.
  - attnT is assembled in PERMUTED token order m' = u*512 + t (u = 8-phase
    index, t = token-within-block), making every y evacuation a contiguous
    [64, 512] copy (no stride-8 scatter). The output projection runs in
    permuted order; the final DMA un-permutes via strided HBM rows
    (row = 8t + u), which costs the same descriptors as a dense store.
"""

import numpy as np
from contextlib import ExitStack

import ml_dtypes
import concourse.bass as bass
import concourse.tile as tile
from concourse import bacc, mybir
from concourse.bass_utils import run_bass_kernel_spmd
from concourse.masks import make_identity

dt = mybir.dt

N_TOK = 4096
C = 512
H = 8
D = 64
TB = 512          # tokens per head block
SUB = TB // 128   # 4 token tiles per head block
CT = C // 128     # 4 contraction tiles
NB = 2 * C + CT + 2 * C          # fused bias row length (2052)
NB_PAD = ((NB + 15) // 16) * 16  # padded to xbar tile rows (2064)

_cache = {}

# Precision knobs (for error-budget experiments; final values tuned on HW).
QK_F32 = False     # keep q,k tiles fp32 and run scores matmuls in fp32
OUT_F32 = False    # keep attnT/wpT in f32r (output-projection path fp32)


def _emit(ctx: ExitStack, tc, out_d, x_d, wqkv_d, wp_d, bias_d):
    nc = tc.nc
    f32, bf16 = dt.float32, dt.bfloat16
    Ident = mybir.ActivationFunctionType.Identity

    const = ctx.enter_context(tc.tile_pool(name="const", bufs=1))
    xtp = ctx.enter_context(tc.tile_pool(name="xtp", bufs=8))      # xT tiles
    headp = ctx.enter_context(tc.tile_pool(name="headp", bufs=2))  # qk
    vtp = ctx.enter_context(tc.tile_pool(name="vtp", bufs=2))      # vT
    smallp = ctx.enter_context(tc.tile_pool(name="smallp", bufs=2))
    outp = ctx.enter_context(tc.tile_pool(name="outp", bufs=3))
    ps_pair = ctx.enter_context(tc.tile_pool(name="ps_pair", bufs=2, space="PSUM"))
    ps_big = ctx.enter_context(tc.tile_pool(name="ps_big", bufs=2, space="PSUM"))
    ps_sm = ctx.enter_context(tc.tile_pool(name="ps_sm", bufs=1, space="PSUM"))

    # ---------------- one-time setup ----------------
    ident = const.tile([64, 64], f32)
    make_identity(nc, ident)

    # Every input load (x, weights, fused biases) goes through the xbar
    # transpose engine — the whole input phase stays in T-mode, so the
    # DmaTranspose<->DmaCopy serialization is paid exactly once, at the
    # switch to the output-store phase.
    # 3D-out semantics: dst[p, blk, f] = src2d[f, blk*128+p].
    # Fused biases first (single small C-mode DMA; everything after it is
    # T-mode until the output-store phase, so the DmaTranspose<->DmaCopy
    # serialization is paid once at each end of the kernel).
    biases = const.tile([128, NB_PAD], bf16)
    nc.sync.dma_start(biases, bias_d)
    qk_bias = biases[:, 0:2 * C]
    vbias = biases[:, 2 * C:2 * C + CT]
    wp_bias = biases[:, 2 * C + CT:2 * C + CT + 2 * C]

    xT0 = xtp.tile([128, CT, TB], bf16, tag="xT")
    nc.sync.dma_start_transpose(xT0, x_d[0:TB, :])

    wqkT = const.tile([128, CT, 2 * C], bf16)    # [p, ct, f] = wqkv[f, ct*128+p]
    for ct in range(CT):
        nc.sync.dma_start_transpose(
            wqkT[:, ct, :], wqkv_d[0:2 * C, ct * 128:(ct + 1) * 128])
    wvT = const.tile([128, CT, C], bf16)         # [p, ci, vc] = wqkv[2C+vc, ci*128+p]
    nc.sync.dma_start_transpose(wvT, wqkv_d[2 * C:3 * C, :])

    # Prefetch every head's xT up front (bufs=8): the transfers pipeline far
    # ahead of the ~16us/head compute, so no head ever waits on its xT and
    # the Sync engine is quiet during the head loop.
    xTs = [xT0]
    for hh in range(1, H):
        xTh = xtp.tile([128, CT, TB], bf16, tag="xT")
        nc.sync.dma_start_transpose(xTh, x_d[hh * TB:(hh + 1) * TB, :])
        xTs.append(xTh)
    wpT = const.tile([128, CT, C], bf16)         # [p, j, f] = wp_w[f, j*128+p]
    # (wpT's transpose-DMA is issued inside the head loop — it is only
    # needed by the output projection.)

    # PE warm-up: ~6us of dummy matmuls on the identity while the first
    # DMAs land, so the HAM clock gate reaches 8/8 before the real stream
    # and stays there (no >3.4us PE idle before the first qk matmul).
    warm = ps_sm.tile([128, 2, C], f32, tag="pss")
    for wi in range(26):
        nc.tensor.matmul(warm[0:64, 0, 0:64], ident, ident,
                         start=True, stop=True)

    # Two block-diag W^T stationaries (per head parity); off-diag zeroed once.
    bd0 = const.tile([128, 128], bf16, tag="bd0")
    bd1 = const.tile([128, 128], bf16, tag="bd1")
    bds = [bd0, bd1]
    for bd in bds:
        nc.vector.memset(bd, 0.0)

    # Transposed attention output in permuted token order:
    # attnT[p, j, u*512 + t] = attn[8t + u, j*128 + p]
    attnT = const.tile([128, CT, N_TOK], dt.float32r if OUT_F32 else bf16)

    # ---------------- per-head pipeline ----------------
    for h in range(H):
        tok0 = h * TB

        xT = xTs[h]
        if h == 2:
            nc.sync.dma_start_transpose(wpT, wp_d)

        # qk[t, f]: stationary xT tiles, moving bf16 weights; psum pairs.
        qk = headp.tile([128, SUB, 2 * C], bf16, tag="qk")
        for i in range(SUB):
            pq = ps_pair.tile([128, 2 * C], f32, tag="pspair")
            for ct in range(CT):
                for g in range(2):
                    nc.tensor.matmul(
                        pq[:, g * C:(g + 1) * C],
                        xT[:, ct, i * 128:(i + 1) * 128],
                        wqkT[:, ct, g * C:(g + 1) * C],
                        start=(ct == 0), stop=(ct == CT - 1))
            nc.vector.tensor_add(qk[:, i, :], pq, qk_bias)

        # scores[d, e] (64x64), bf16 single-pass, accumulated over 4 i x 8 s.
        # Col-packed: even-s terms accumulate in PSUM rows 0:64 (col groups
        # 0-1), odd-s in rows 64:128 (col groups 2-3) — the two matmul
        # streams run concurrently in disjoint column halves of the PE array.
        # (Even/odd accumulators live in different PSUM banks AND different
        # partition halves so the start=True bank-clears stay disjoint.)
        sc = ps_sm.tile([128, 2, C], f32, tag="pss")
        sc_e = sc[0:64, 0, 0:64]
        sc_o = sc[64:128, 1, 0:64]
        npair = SUB * (H // 2)
        k = 0
        for i in range(SUB):
            for sp in range(H // 2):
                s0, s1 = 2 * sp, 2 * sp + 1
                nc.tensor.matmul(
                    sc_e,
                    qk[:, i, s0 * D:(s0 + 1) * D],
                    qk[:, i, C + s0 * D: C + (s0 + 1) * D],
                    start=(k == 0), stop=(k == npair - 1),
                    tile_position=(0, 0))
                nc.tensor.matmul(
                    sc_o,
                    qk[:, i, s1 * D:(s1 + 1) * D],
                    qk[:, i, C + s1 * D: C + (s1 + 1) * D],
                    start=(k == 0), stop=(k == npair - 1),
                    tile_position=(0, 64))
                k += 1
        sco = smallp.tile([64, 64], f32, tag="sco")
        nc.vector.tensor_copy(sco, sc_o)
        scf = smallp.tile([64, 64], f32, tag="scf")
        nc.vector.tensor_add(scf, sc_e, sco)

        # vT[vc, t] = v[tok0+t, vc], +bias, bf16.
        vT = vtp.tile([128, CT, TB], bf16, tag="vT")
        for ct in range(CT):
            pv = ps_big.tile([128, TB], f32, tag="ps")
            for ci in range(CT):
                nc.tensor.matmul(
                    pv,
                    wvT[:, ci, ct * 128:(ct + 1) * 128],
                    xT[:, ci, :],
                    start=(ci == 0), stop=(ci == CT - 1))
            nc.scalar.activation(vT[:, ct, :], pv, Ident,
                                 bias=vbias[:, ct:ct + 1])

        # softmax over e (free axis); scale 1/sqrt(64) folded into exp.
        rmax = smallp.tile([64, 1], f32, tag="rmax")
        nc.vector.reduce_max(rmax, scf, axis=mybir.AxisListType.X)
        ebias = smallp.tile([64, 1], f32, tag="ebias")
        nc.vector.tensor_scalar_mul(ebias, rmax, -0.125)
        wexp = smallp.tile([64, 64], f32, tag="wexp")
        nc.scalar.activation(wexp, scf, mybir.ActivationFunctionType.Exp,
                             bias=ebias, scale=0.125)
        rsum = smallp.tile([64, 1], f32, tag="rsum")
        nc.vector.reduce_sum(rsum, wexp, axis=mybir.AxisListType.X)
        rrec = smallp.tile([64, 1], f32, tag="rrec")
        nc.vector.reciprocal(rrec, rsum)
        wn = smallp.tile([64, 64], f32, tag="wn")
        nc.vector.tensor_scalar_mul(wn, wexp, rrec)

        # block-diag(W^T): diag blocks at [0:64,0:64] and [64:128,64:128].
        bd = bds[h % 2]
        wps = ps_sm.tile([64, 64], f32, tag="pss")
        nc.tensor.transpose(wps, wn, ident)
        nc.vector.tensor_copy(bd[0:64, 0:64], wps)
        nc.vector.tensor_copy(bd[64:128, 64:128], wps)

        # y: psum rows (sl*64+d), cols t; tokens m = 8t + (2*tau + sl).
        # Contiguous writes into permuted attnT columns u*512 + t.
        j = h // 2
        pb = (h % 2) * 64
        for tau in range(CT):
            py = ps_big.tile([128, TB], f32, tag="ps")
            nc.tensor.matmul(py, bd, vT[:, tau, :], start=True, stop=True)
            u0 = 2 * tau
            if tau % 2 == 0:
                nc.vector.tensor_copy(
                    attnT[pb:pb + 64, j, u0 * TB:(u0 + 1) * TB], py[0:64, :])
                nc.vector.tensor_copy(
                    attnT[pb:pb + 64, j, (u0 + 1) * TB:(u0 + 2) * TB],
                    py[64:128, :])
            else:
                nc.scalar.activation(
                    attnT[pb:pb + 64, j, u0 * TB:(u0 + 1) * TB], py[0:64, :],
                    Ident)
                nc.scalar.activation(
                    attnT[pb:pb + 64, j, (u0 + 1) * TB:(u0 + 2) * TB],
                    py[64:128, :], Ident)

    # ---------------- output projection (permuted order) ----------------
    # Column block mt covers permuted tokens [mt*128, (mt+1)*128):
    # u = mt // 4, t = (mt % 4)*128 + q  ->  HBM row 8t + u.
    for mp in range(N_TOK // 256):          # pairs of mt tiles
        pp = ps_pair.tile([128, 2 * C], f32, tag="pspair")
        for half in range(2):
            mt = 2 * mp + half
            for j in range(CT):
                nc.tensor.matmul(
                    pp[:, half * C:(half + 1) * C],
                    attnT[:, j, mt * 128:(mt + 1) * 128],
                    wpT[:, j, :],
                    start=(j == 0), stop=(j == CT - 1))
        ob = outp.tile([128, 2 * C], bf16, tag="ob")
        nc.vector.tensor_add(ob, pp, wp_bias)
        for half in range(2):
            mt = 2 * mp + half
            u, tblk = mt // 4, mt % 4
            row0 = 8 * tblk * 128 + u
            nc.sync.dma_start(
                out_d[row0:row0 + 8 * 127 + 1:8, :],
                ob[:, half * C:(half + 1) * C])


def _build():
    nc = bacc.Bacc("TRN2", target_bir_lowering=False, debug=False,
                   num_devices=8)
    x_d = nc.dram_tensor("xbf", [N_TOK, C], dt.bfloat16,
                         kind="ExternalInput").ap()
    wqkv_d = nc.dram_tensor("wqkv", [3 * C, C], dt.bfloat16,
                            kind="ExternalInput").ap()
    wp_d = nc.dram_tensor("wp", [C, C], dt.bfloat16,
                          kind="ExternalInput").ap()
    bias_d = nc.dram_tensor("biasf", [128, NB_PAD], dt.bfloat16,
                            kind="ExternalInput").ap()
    out_d = nc.dram_tensor("out", [N_TOK, C], dt.bfloat16,
                           kind="ExternalOutput").ap()

    with tile.TileContext(nc) as tc:
        with ExitStack() as ctx:
            _emit(ctx, tc, out_d, x_d, wqkv_d, wp_d, bias_d)
    nc.compile()
    return nc


def _get_nc():
    if "nc" not in _cache:
        _cache["nc"] = _build()
    return _cache["nc"]


def _prep_weights(wqkv_w, wqkv_b, wp_w, wp_b):
    wqkv_w = np.asarray(wqkv_w, np.float32)
    wqkv_b = np.asarray(wqkv_b, np.float32)
    wp_w = np.asarray(wp_w, np.float32)
    wp_b = np.asarray(wp_b, np.float32)
    bf = ml_dtypes.bfloat16
    # Weights ship raw (bf16-cast only) — the xbar transpose DMA produces
    # the c-major SBUF layouts on device.
    wqkv = np.ascontiguousarray(wqkv_w).astype(bf)
    wp = np.ascontiguousarray(wp_w).astype(bf)
    # Fused bias row: [qk bias (1024) | v bias (4, partitioned) | wp bias x2]
    qkb = np.broadcast_to(wqkv_b[None, :2 * C], (128, 2 * C))
    vb = wqkv_b[2 * C:].reshape(CT, 128).T
    wpb = np.broadcast_to(np.tile(wp_b, 2)[None, :], (128, 2 * C))
    biasf = np.zeros((128, NB_PAD), np.float32)
    biasf[:, :NB] = np.concatenate([qkb, vb, wpb], axis=1)
    return {"wqkv": wqkv, "wp": wp,
            "biasf": np.ascontiguousarray(biasf).astype(bf)}


def kernel(x, wqkv_w, wqkv_b, wp_w, wp_b, _trace=False, **_trace_kwargs):
    nc = _get_nc()
    x = np.asarray(x, dtype=np.float32).astype(ml_dtypes.bfloat16)
    w = _prep_weights(wqkv_w, wqkv_b, wp_w, wp_b)
    in_maps = [dict(w, xbf=np.ascontiguousarray(x[i])) for i in range(8)]
    res = run_bass_kernel_spmd(nc, in_maps, list(range(8)),
                               trace=_trace, **_trace_kwargs)
    out = np.stack([r["out"] for r in res.results], axis=0).astype(np.float32)
    if _trace:
        return out, res
    return out
